# revision 28
# baseline (speedup 1.0000x reference)
"""Causal single-head attention (B=4, S=4096, D=1024, Dk=128) on 8 TRN2 NeuronCores.

Sharding: 4 batches x 2 cores/batch. Per batch the 16 query supertiles (256
rows) interleave across the core pair (slot t hosts supertile j = 2t+1-g for
core group g), so every core executes the identical instruction graph (SPMD)
with the causal workload balanced; per-core variation (rows, padding, masks)
is carried by the input data.

On-chip layout is fully transposed: projections give Q^T/K^T/V^T with dk on
partitions; scores are computed as S^T = kt.T @ qt so the softmax'd tile is
already P^T (keys on partitions), feeding P@V (lhsT = V rows, rhs = P^T) with
no per-block transposes on the attention path.  V rows come from 4 PE
transposes per chunk.

Optimizations vs the 191us baseline:
- exp batched 4 key blocks per ACTIVATE ([128,1024] across 2 PSUM banks):
  amortizes the ~352-cycle ACT fixed cost (144 calls -> 36).
- PE queue software-pipelined: S-matmul groups issue 2 groups ahead of the
  PV/rowsum matmuls, and chunk c+1's projection matmuls interleave into slot
  c's steps, so the PE never idles waiting on the scalar-engine exp.
- normalization moved to host: kernel emits unnormalized O^T plus packed
  row-sums; host divides, adds the V bias (O += l*bv trick), transposes, and
  zeroes dead rows.  Removes on-chip reciprocal/broadcast/transpose chains.
- causal masks are 4 static [128,256] 0/1 patterns (host-supplied), one
  bf16 multiply per slot.
- ~44 junk matmuls warm the PE HAM clock gate (1.2 -> 2.4 GHz) during the
  initial x^T DMA; a dummy exp preloads the ACT spline table.
"""

import numpy as np
import ml_dtypes

import concourse.bass as bass
import concourse.mybir as mybir
import concourse.tile as tile
from concourse import bacc
from concourse.bass_utils import run_bass_kernel_spmd
from concourse.masks import make_identity

F32 = mybir.dt.float32
BF16 = mybir.dt.bfloat16
AF = mybir.ActivationFunctionType
ALU = mybir.AluOpType

B, S, D, DK = 4, 4096, 1024, 128
NSLOT = 8          # static slots per core
STR = 256          # supertile rows (1 slot = 256 queries)
NKB = 32           # key blocks per batch
NCORE = 8
GRP = 4            # key blocks per exp group (one ACTIVATE)
SCALE = float(1.0 / np.sqrt(np.float32(DK)))
BF = ml_dtypes.bfloat16


def build_graph():
    nc = bacc.Bacc("TRN2", target_bir_lowering=False, debug=False, num_devices=NCORE)

    # xt layout: [partition(D%128), chunk, D-subchunk(mc), col] so one chunk is
    # a single contiguous 8KB run per partition on both DMA sides (full HBM BW)
    xt_e = nc.declare_dram_parameter("xt", [128, 8, 8, 512], BF16, isOutput=False)
    wq_e = nc.declare_dram_parameter("wq", [128, 8, DK], BF16, isOutput=False)
    wk_e = nc.declare_dram_parameter("wk", [128, 8, DK], BF16, isOutput=False)
    wv_e = nc.declare_dram_parameter("wv", [128, 8, DK], BF16, isOutput=False)
    bq_e = nc.declare_dram_parameter("bq", [128, 1], F32, isOutput=False)
    pmk_e = nc.declare_dram_parameter("pmk", [128, NKB], BF16, isOutput=False)
    tri_e = nc.declare_dram_parameter("tri", [128, GRP * STR], BF16, isOutput=False)
    ot_e = nc.declare_dram_parameter("ot", [NSLOT, DK, STR], F32, isOutput=True)
    lb_e = nc.declare_dram_parameter("lb", [NSLOT, 4, STR], F32, isOutput=True)

    with tile.TileContext(nc) as tc:
        with (
            tc.tile_pool(name="const", bufs=1) as const,
            tc.tile_pool(name="big", bufs=1) as big,
            tc.tile_pool(name="vtmp", bufs=2) as vtmp,
            tc.tile_pool(name="ptp", bufs=3) as ptp,
            tc.tile_pool(name="lbp", bufs=4) as lbp,
            tc.tile_pool(name="osbp", bufs=4) as osbp,
            tc.tile_pool(name="projp", bufs=2, space="PSUM") as projp,
            tc.tile_pool(name="sp", bufs=2, space="PSUM") as sp,
            tc.tile_pool(name="op", bufs=1, space="PSUM") as op,
            tc.tile_pool(name="ltp", bufs=1, space="PSUM") as ltp,
        ):
            # ---- constants / small inputs ----
            # x^T chunks hog the sync queue (first in line); small inputs ride
            # the otherwise-idle scalar queue; outputs go out via gpsimd.
            ident = const.tile([128, 128], F32)
            make_identity(nc, ident)

            xt_sb = big.tile([128, 8, 8, 512], BF16)
            kt_sb = big.tile([128, NKB, DK], BF16)
            v_sb = big.tile([128, NKB, DK], BF16)
            qt_sb = big.tile([128, NSLOT, STR], BF16)

            # All x^T chunks ride the sync HWDGE ring in order: per-core HBM
            # supply during the 8-core startup burst is ~140 GB/s no matter
            # how many rings pull, so a serial earliest-first stream is
            # rate-matched to slot consumption (striping makes chunk 0 slower).
            # Chunk 0 goes in halves so the K projection starts ~3us earlier.
            nc.sync.dma_start(xt_sb[:, 0, 0:4], xt_e[:, 0, 0:4])
            nc.sync.dma_start(xt_sb[:, 0, 4:8], xt_e[:, 0, 4:8])
            for c in range(1, 8):
                nc.sync.dma_start(xt_sb[:, c], xt_e[:, c])

            # HAM warmup: junk matmuls gated only on a trivial early memset
            # start almost as soon as the PE queue is ready, so the clock
            # gate opens before real work and never re-throttles.
            junk = const.tile([128, 128], BF16)
            nc.vector.memset(junk, 0.0)
            warm = projp.tile([128, 512], F32, tag="proj")
            for _ in range(36):
                nc.tensor.matmul(
                    warm[:, :128], lhsT=junk, rhs=junk,
                    start=True, stop=True,
                )

            ident_bf = const.tile([128, 128], BF16)
            nc.vector.tensor_copy(ident_bf, ident)

            # weight order = consumption order: K proj, then V, then Q
            wk_sb = const.tile([128, 8, DK], BF16)
            wq_sb = const.tile([128, 8, DK], BF16)
            wv_sb = const.tile([128, 8, DK], BF16)
            bq_sb = const.tile([128, 1], F32)
            pmk_sb = const.tile([128, NKB], BF16)
            tri_sb = const.tile([128, GRP * STR], BF16)
            nc.scalar.dma_start(wk_sb[:], wk_e[:])
            nc.scalar.dma_start(wv_sb[:], wv_e[:])
            nc.scalar.dma_start(wq_sb[:], wq_e[:])
            nc.scalar.dma_start(pmk_sb[:], pmk_e[:])
            nc.scalar.dma_start(bq_sb[:], bq_e[:])
            nc.scalar.dma_start(tri_sb[:], tri_e[:])
            scratch = const.tile([128, 1], F32)

            # ACT spline-table preload (~2.7us, off the critical path)
            nc.scalar.activation(scratch, bq_sb, AF.Exp)

            def proj_K(c):
                kps = projp.tile([128, 512], F32, tag="proj", name=f"kps{c}")
                for mc in range(8):
                    nc.tensor.matmul(
                        kps,
                        lhsT=wk_sb[:, mc],
                        rhs=xt_sb[:, c, mc, :],
                        start=(mc == 0),
                        stop=(mc == 7),
                    )
                # NOTE: bk dropped on purpose — a key-side bias shifts every
                # score in a softmax row equally and cancels exactly.
                nc.vector.tensor_copy(kt_sb[:, 4 * c : 4 * (c + 1), :], kps)

            def proj_V(c):
                vps = projp.tile([128, 512], F32, tag="proj", name=f"vps{c}")
                for mc in range(8):
                    nc.tensor.matmul(
                        vps,
                        lhsT=wv_sb[:, mc],
                        rhs=xt_sb[:, c, mc, :],
                        start=(mc == 0),
                        stop=(mc == 7),
                    )
                # NOTE: bv dropped on chip — host adds it back (sum(P^)=1).
                vt = vtmp.tile([128, 512], BF16, tag="vt", name=f"vt{c}")
                nc.vector.tensor_copy(vt, vps)
                # V^T -> V rows (4 PE transposes), key-padding folded into
                # the single evacuation multiply.
                tp = ltp.tile([128, GRP, 128], BF16, tag="lt", name=f"tp{c}")
                for i in range(GRP):
                    nc.tensor.transpose(
                        tp[:, i, :], vt[:, 128 * i : 128 * (i + 1)], ident_bf
                    )
                nc.vector.tensor_tensor(
                    v_sb[:, 4 * c : 4 * (c + 1), :],
                    tp,
                    pmk_sb[:, 4 * c : 4 * (c + 1)].to_broadcast([128, GRP, DK]),
                    ALU.mult,
                )

            def proj_Q_work(c):
                """Return 8 closures (one matmul each); the last also emits
                the bias-add evacuation to qt_sb."""
                box = {}

                def mk(mc):
                    def go():
                        if mc == 0:
                            box["qps"] = projp.tile(
                                [128, 512], F32, tag="proj", name=f"qps{c}"
                            )
                        nc.tensor.matmul(
                            box["qps"][:, :STR],
                            lhsT=wq_sb[:, mc],
                            rhs=xt_sb[:, c, mc, STR:],
                            start=(mc == 0),
                            stop=(mc == 7),
                        )
                        if mc == 7:
                            nc.vector.tensor_tensor(
                                qt_sb[:, c, :],
                                box["qps"][:, :STR],
                                bq_sb[:].to_broadcast([128, STR]),
                                ALU.add,
                            )
                    return go

                return [mk(mc) for mc in range(8)]

            # ---- prologue: chunk 0 projections ----
            proj_K(0)
            proj_V(0)
            for go in proj_Q_work(0):
                go()

            # ---- main loop: slot c consumes chunks 0..c; chunk c+1's
            # projections interleave into slot c's PE stream ----
            for c in range(NSLOT):
                ngrp = c + 1
                nxt = c + 1 if c + 1 < NSLOT else None
                pts = {}

                def emit_S(g, c=c, pts=pts):
                    s_t = sp.tile([128, GRP * STR], F32, tag="s", name=f"s{c}_{g}")
                    for i in range(GRP):
                        nc.tensor.matmul(
                            s_t[:, STR * i : STR * (i + 1)],
                            lhsT=kt_sb[:, GRP * g + i, :],
                            rhs=qt_sb[:, c, :],
                            start=True,
                            stop=True,
                        )
                    pt = ptp.tile([128, GRP * STR], BF16, tag="pt", name=f"pt{c}_{g}")
                    nc.scalar.activation(pt, s_t, AF.Exp, scale=SCALE)
                    if g == c:
                        # diagonal group: static causal mask (kills the two
                        # future blocks entirely on g=1 cores)
                        nc.vector.tensor_tensor(pt, pt, tri_sb, ALU.mult)
                    pts[g] = pt

                # Process the diagonal group first: its exp -> mask-multiply
                # latency then hides behind the other groups' matmuls instead
                # of stalling the slot's tail.
                g_seq = [c] + list(range(c)) if ngrp > 1 else [0]
                emit_S(g_seq[0])
                if nxt is not None:
                    proj_K(nxt)
                if ngrp >= 2:
                    emit_S(g_seq[1])
                if nxt is not None:
                    proj_V(nxt)
                qwork = proj_Q_work(nxt) if nxt is not None else []
                qi = 0
                o_t = op.tile([128, STR], F32, tag="o", name=f"o{c}")
                l_t = ltp.tile([128, STR], F32, tag="lt", name=f"l{c}")
                # The packs only write rows 0/32/64/96 and the slot shares its
                # PSUM bytes with the V-transpose staging tile, whose matmuls
                # leave has_written set — zero the data so the start=False
                # packs accumulate onto 0 rather than stale transpose bits.
                nc.vector.memset(l_t, 0.0)
                for k in range(ngrp):
                    if k + 2 < ngrp:
                        emit_S(g_seq[k + 2])
                    for _ in range(min(2, len(qwork) - qi)):
                        qwork[qi]()
                        qi += 1
                    g = g_seq[k]
                    pt = pts[g]
                    for i in range(GRP):
                        nc.tensor.matmul(
                            o_t,
                            lhsT=v_sb[:, GRP * g + i, :],
                            rhs=pt[:, STR * i : STR * (i + 1)],
                            start=(k == 0 and i == 0),
                            stop=(k == ngrp - 1 and i == GRP - 1),
                        )
                    # 4 row-sum matmuls packed into the 4 column groups of the
                    # PE array (concurrent on separate XBUSes)
                    for j in range(GRP):
                        nc.tensor.matmul(
                            l_t[32 * j : 32 * j + 1, :],
                            lhsT=pmk_sb[:, GRP * g + j : GRP * g + j + 1],
                            rhs=pt[:, STR * j : STR * (j + 1)],
                            start=(k == 0 and j == 0),
                            stop=(k == ngrp - 1 and j == GRP - 1),
                            tile_position=(0, 32 * j),
                            skip_group_check=True,
                        )
                while qi < len(qwork):
                    qwork[qi]()
                    qi += 1
                # evacuate unnormalized O^T and the 4 packed row-sum rows
                o_sb = osbp.tile([128, STR], F32, tag="osb", name=f"osb{c}")
                nc.vector.tensor_copy(o_sb, o_t)
                nc.gpsimd.dma_start(ot_e[c], o_sb)
                l_big = lbp.tile([128, STR], F32, tag="lbig", name=f"lbig{c}")
                nc.vector.tensor_copy(l_big, l_t)
                nc.gpsimd.dma_start(lb_e[c], l_big[0:128:32, :])

    nc.compile()
    return nc


def _tri_masks():
    """T1[k,q] = k<=q, T2[k,q] = k+128<=q for the diagonal supertile."""
    k = np.arange(128)[:, None]
    q = np.arange(STR)[None, :]
    t1 = (k <= q).astype(np.float32)
    t2 = (k + 128 <= q).astype(np.float32)
    return t1, t2


def shard_inputs(x, padding_mask, Wq, bq, Wk, bk, Wv, bv):
    """Build per-core in_maps plus the info needed to gather on the host."""
    x = np.asarray(x, np.float32)
    pm = np.asarray(padding_mask, np.float32)
    w_tiles = {}
    for name, W in (("wq", Wq), ("wk", Wk), ("wv", Wv)):
        w_tiles[name] = np.ascontiguousarray(
            np.asarray(W, np.float32).reshape(8, 128, DK).transpose(1, 0, 2)
        ).astype(BF)
    bq_t = np.ascontiguousarray(np.asarray(bq, np.float32).reshape(128, 1))
    bv_f = np.asarray(bv, np.float32)
    t1, t2 = _tri_masks()
    ones = np.ones((128, STR), np.float32)
    zeros = np.zeros((128, STR), np.float32)
    tri_g = {
        0: np.concatenate([ones, ones, t1, t2], axis=1).astype(BF),
        1: np.concatenate([zeros, zeros, t1, t2], axis=1).astype(BF),
    }
    in_maps = []
    row_maps = []
    base = np.arange(S).reshape(8, 2, STR)
    for cc in range(NCORE):
        b, g = cc % 4, cc // 4
        perm = (base[:, ::-1, :] if g == 1 else base).reshape(-1)
        xp = x[b][perm]                       # [S, D] permuted rows
        # [128, mc, pos] -> [128, chunk, mc, 512] (chunk-contiguous per partition)
        xt3 = xp.T.reshape(8, 128, S).transpose(1, 0, 2)
        xt = np.ascontiguousarray(
            xt3.reshape(128, 8, 8, 512).swapaxes(1, 2)
        ).astype(BF)
        qrows = perm.reshape(8, 2, STR)[:, 1, :]   # own rows per slot [8, 256]
        alive = pm[b][qrows].astype(np.float32)    # [8, 256]
        pmk = pm[b][perm].reshape(NKB, 128).T      # [128, 32]
        in_maps.append({
            "xt": xt,
            **w_tiles,
            "bq": bq_t,
            "pmk": np.ascontiguousarray(pmk).astype(BF),
            "tri": np.ascontiguousarray(tri_g[g]),
        })
        row_maps.append((b, qrows, alive, bv_f))
    return in_maps, row_maps


def gather_outputs(results, row_maps):
    full = np.zeros((B, S, DK), np.float32)
    for cc in range(NCORE):
        b, qrows, alive, bv_f = row_maps[cc]
        ot = np.asarray(results[cc]["ot"], np.float32)   # [8, 128, 256] O^T
        lb = np.asarray(results[cc]["lb"], np.float32)   # [8, 4, 256]
        l = lb.sum(axis=1)                               # [8, 256]
        denom = l + (alive <= 0.0)                       # dead rows: avoid /0
        o = ot.transpose(0, 2, 1) / denom[..., None] + bv_f
        o *= alive[..., None]
        full[b, qrows] = o
    return full


_NC_CACHE = None


def _get_graph():
    global _NC_CACHE
    if _NC_CACHE is None:
        _NC_CACHE = build_graph()
    return _NC_CACHE


def kernel(x, padding_mask, Wq, bq, Wk, bk, Wv, bv):
    nc = _get_graph()
    in_maps, row_maps = shard_inputs(x, padding_mask, Wq, bq, Wk, bk, Wv, bv)
    res = run_bass_kernel_spmd(nc, in_maps, core_ids=list(range(NCORE)))
    return gather_outputs(res.results, row_maps)


# revision 30
# speedup vs baseline: 1.1084x; 1.1084x over previous
"""Causal single-head attention (B=4, S=4096, D=1024, Dk=128) on 8 TRN2 NeuronCores.

Sharding: 4 batches x 2 cores/batch. Per batch the 16 query supertiles (256
rows) interleave across the core pair (slot t hosts supertile j = 2t+1-g for
core group g), so every core executes the identical instruction graph (SPMD)
with the causal workload balanced; per-core variation (rows, padding, masks)
is carried by the input data.

On-chip layout is fully transposed: projections give Q^T/K^T/V^T with dk on
partitions; scores are computed as S^T = kt.T @ qt so the softmax'd tile is
already P^T (keys on partitions), feeding P@V (lhsT = V rows, rhs = P^T) with
no per-block transposes on the attention path.  V rows come from 4 PE
transposes per chunk.

Optimizations vs the 191us baseline:
- exp batched 4 key blocks per ACTIVATE ([128,1024] across 2 PSUM banks):
  amortizes the ~352-cycle ACT fixed cost (144 calls -> 36).
- PE queue software-pipelined: S-matmul groups issue 2 groups ahead of the
  PV/rowsum matmuls, and chunk c+1's projection matmuls interleave into slot
  c's steps, so the PE never idles waiting on the scalar-engine exp.
- normalization moved to host: kernel emits unnormalized O^T plus packed
  row-sums; host divides, adds the V bias (O += l*bv trick), transposes, and
  zeroes dead rows.  Removes on-chip reciprocal/broadcast/transpose chains.
- causal masks are 4 static [128,256] 0/1 patterns (host-supplied), one
  bf16 multiply per slot.
- ~44 junk matmuls warm the PE HAM clock gate (1.2 -> 2.4 GHz) during the
  initial x^T DMA; a dummy exp preloads the ACT spline table.
"""

import numpy as np
import ml_dtypes

import concourse.bass as bass
import concourse.mybir as mybir
import concourse.tile as tile
from concourse import bacc
from concourse.bass_utils import run_bass_kernel_spmd
from concourse.masks import make_identity

F32 = mybir.dt.float32
BF16 = mybir.dt.bfloat16
AF = mybir.ActivationFunctionType
ALU = mybir.AluOpType

B, S, D, DK = 4, 4096, 1024, 128
NSLOT = 8          # static slots per core
STR = 256          # supertile rows (1 slot = 256 queries)
NKB = 32           # key blocks per batch
NCORE = 8
GRP = 4            # key blocks per exp group (one ACTIVATE)
SCALE = float(1.0 / np.sqrt(np.float32(DK)))
BF = ml_dtypes.bfloat16


def build_graph():
    nc = bacc.Bacc("TRN2", target_bir_lowering=False, debug=False, num_devices=NCORE)

    # xt layout: [partition(D%128), chunk, D-subchunk(mc), col] so one chunk is
    # a single contiguous 8KB run per partition on both DMA sides (full HBM BW)
    xt_e = nc.declare_dram_parameter("xt", [128, 8, 8, 512], BF16, isOutput=False)
    wq_e = nc.declare_dram_parameter("wq", [128, 8, DK], BF16, isOutput=False)
    wk_e = nc.declare_dram_parameter("wk", [128, 8, DK], BF16, isOutput=False)
    wv_e = nc.declare_dram_parameter("wv", [128, 8, DK], BF16, isOutput=False)
    bq_e = nc.declare_dram_parameter("bq", [128, 1], F32, isOutput=False)
    pmk_e = nc.declare_dram_parameter("pmk", [128, NKB], BF16, isOutput=False)
    tri_e = nc.declare_dram_parameter("tri", [128, GRP * STR], BF16, isOutput=False)
    ot_e = nc.declare_dram_parameter("ot", [NSLOT, DK, STR], F32, isOutput=True)
    lb_e = nc.declare_dram_parameter("lb", [NSLOT, 4, STR], F32, isOutput=True)

    with tile.TileContext(nc) as tc:
        with (
            tc.tile_pool(name="const", bufs=1) as const,
            tc.tile_pool(name="big", bufs=1) as big,
            tc.tile_pool(name="vtmp", bufs=2) as vtmp,
            tc.tile_pool(name="ptp", bufs=3) as ptp,
            tc.tile_pool(name="lbp", bufs=4) as lbp,
            tc.tile_pool(name="osbp", bufs=4) as osbp,
            tc.tile_pool(name="projp", bufs=2, space="PSUM") as projp,
            tc.tile_pool(name="sp", bufs=2, space="PSUM") as sp,
            tc.tile_pool(name="op", bufs=1, space="PSUM") as op,
            tc.tile_pool(name="ltp", bufs=1, space="PSUM") as ltp,
        ):
            # ---- constants / small inputs ----
            # x^T chunks hog the sync queue (first in line); small inputs ride
            # the otherwise-idle scalar queue; outputs go out via gpsimd.
            ident = const.tile([128, 128], F32)
            make_identity(nc, ident)

            xt_sb = big.tile([128, 8, 8, 512], BF16)
            kt_sb = big.tile([128, NKB, DK], BF16)
            v_sb = big.tile([128, NKB, DK], BF16)
            qt_sb = big.tile([128, NSLOT, STR], BF16)

            # All x^T chunks ride the sync HWDGE ring in order: per-core HBM
            # supply during the 8-core startup burst is ~140 GB/s no matter
            # how many rings pull, so a serial earliest-first stream is
            # rate-matched to slot consumption (striping makes chunk 0 slower).
            for c in range(8):
                nc.sync.dma_start(xt_sb[:, c], xt_e[:, c])

            # HAM warmup: junk matmuls gated only on a trivial early memset
            # start almost as soon as the PE queue is ready, so the clock
            # gate opens before real work and never re-throttles.
            junk = const.tile([128, 128], BF16)
            nc.vector.memset(junk, 0.0)
            warm = projp.tile([128, 512], F32, tag="proj")
            for _ in range(64):
                nc.tensor.matmul(
                    warm[:, :128], lhsT=junk, rhs=junk,
                    start=True, stop=True,
                )

            ident_bf = const.tile([128, 128], BF16)
            nc.vector.tensor_copy(ident_bf, ident)

            # weight order = consumption order: K proj, then V, then Q
            wk_sb = const.tile([128, 8, DK], BF16)
            wq_sb = const.tile([128, 8, DK], BF16)
            wv_sb = const.tile([128, 8, DK], BF16)
            bq_sb = const.tile([128, 1], F32)
            pmk_sb = const.tile([128, NKB], BF16)
            tri_sb = const.tile([128, GRP * STR], BF16)
            nc.scalar.dma_start(wk_sb[:], wk_e[:])
            nc.scalar.dma_start(wv_sb[:], wv_e[:])
            nc.scalar.dma_start(wq_sb[:], wq_e[:])
            nc.scalar.dma_start(pmk_sb[:], pmk_e[:])
            nc.scalar.dma_start(bq_sb[:], bq_e[:])
            nc.scalar.dma_start(tri_sb[:], tri_e[:])
            scratch = const.tile([128, 1], F32)

            # ACT spline-table preload (~2.7us, off the critical path)
            nc.scalar.activation(scratch, bq_sb, AF.Exp)

            def proj_K(c):
                kps = projp.tile([128, 512], F32, tag="proj", name=f"kps{c}")
                for mc in range(8):
                    nc.tensor.matmul(
                        kps,
                        lhsT=wk_sb[:, mc],
                        rhs=xt_sb[:, c, mc, :],
                        start=(mc == 0),
                        stop=(mc == 7),
                    )
                # NOTE: bk dropped on purpose — a key-side bias shifts every
                # score in a softmax row equally and cancels exactly.
                nc.vector.tensor_copy(kt_sb[:, 4 * c : 4 * (c + 1), :], kps)

            def proj_V(c):
                vps = projp.tile([128, 512], F32, tag="proj", name=f"vps{c}")
                for mc in range(8):
                    nc.tensor.matmul(
                        vps,
                        lhsT=wv_sb[:, mc],
                        rhs=xt_sb[:, c, mc, :],
                        start=(mc == 0),
                        stop=(mc == 7),
                    )
                # NOTE: bv dropped on chip — host adds it back (sum(P^)=1).
                vt = vtmp.tile([128, 512], BF16, tag="vt", name=f"vt{c}")
                nc.vector.tensor_copy(vt, vps)
                # V^T -> V rows (4 PE transposes), key-padding folded into
                # the single evacuation multiply.
                tp = ltp.tile([128, GRP, 128], BF16, tag="lt", name=f"tp{c}")
                for i in range(GRP):
                    nc.tensor.transpose(
                        tp[:, i, :], vt[:, 128 * i : 128 * (i + 1)], ident_bf
                    )
                nc.vector.tensor_tensor(
                    v_sb[:, 4 * c : 4 * (c + 1), :],
                    tp,
                    pmk_sb[:, 4 * c : 4 * (c + 1)].to_broadcast([128, GRP, DK]),
                    ALU.mult,
                )

            def proj_Q_work(c):
                """Return 8 closures (one matmul each); the last also emits
                the bias-add evacuation to qt_sb."""
                box = {}

                def mk(mc):
                    def go():
                        if mc == 0:
                            box["qps"] = projp.tile(
                                [128, 512], F32, tag="proj", name=f"qps{c}"
                            )
                        nc.tensor.matmul(
                            box["qps"][:, :STR],
                            lhsT=wq_sb[:, mc],
                            rhs=xt_sb[:, c, mc, STR:],
                            start=(mc == 0),
                            stop=(mc == 7),
                        )
                        if mc == 7:
                            nc.vector.tensor_tensor(
                                qt_sb[:, c, :],
                                box["qps"][:, :STR],
                                bq_sb[:].to_broadcast([128, STR]),
                                ALU.add,
                            )
                    return go

                return [mk(mc) for mc in range(8)]

            # ---- prologue: chunk 0 projections ----
            proj_K(0)
            proj_V(0)
            for go in proj_Q_work(0):
                go()

            # ---- main loop: slot c consumes chunks 0..c; chunk c+1's
            # projections interleave into slot c's PE stream ----
            for c in range(NSLOT):
                ngrp = c + 1
                nxt = c + 1 if c + 1 < NSLOT else None
                pts = {}

                def emit_S(g, c=c, pts=pts):
                    s_t = sp.tile([128, GRP * STR], F32, tag="s", name=f"s{c}_{g}")
                    for i in range(GRP):
                        nc.tensor.matmul(
                            s_t[:, STR * i : STR * (i + 1)],
                            lhsT=kt_sb[:, GRP * g + i, :],
                            rhs=qt_sb[:, c, :],
                            start=True,
                            stop=True,
                        )
                    pt = ptp.tile([128, GRP * STR], BF16, tag="pt", name=f"pt{c}_{g}")
                    nc.scalar.activation(pt, s_t, AF.Exp, scale=SCALE)
                    if g == c:
                        # diagonal group: static causal mask (kills the two
                        # future blocks entirely on g=1 cores)
                        nc.vector.tensor_tensor(pt, pt, tri_sb, ALU.mult)
                    pts[g] = pt

                # Process the diagonal group first: its exp -> mask-multiply
                # latency then hides behind the other groups' matmuls instead
                # of stalling the slot's tail.
                g_seq = [c] + list(range(c)) if ngrp > 1 else [0]
                emit_S(g_seq[0])
                if nxt is not None:
                    proj_K(nxt)
                if ngrp >= 2:
                    emit_S(g_seq[1])
                if nxt is not None:
                    proj_V(nxt)
                qwork = proj_Q_work(nxt) if nxt is not None else []
                qi = 0
                o_t = op.tile([128, STR], F32, tag="o", name=f"o{c}")
                l_t = ltp.tile([128, STR], F32, tag="lt", name=f"l{c}")
                # The packs only write rows 0/32/64/96 and the slot shares its
                # PSUM bytes with the V-transpose staging tile, whose matmuls
                # leave has_written set — zero the data so the start=False
                # packs accumulate onto 0 rather than stale transpose bits.
                nc.vector.memset(l_t, 0.0)
                for k in range(ngrp):
                    if k + 2 < ngrp:
                        emit_S(g_seq[k + 2])
                    for _ in range(min(2, len(qwork) - qi)):
                        qwork[qi]()
                        qi += 1
                    g = g_seq[k]
                    pt = pts[g]
                    for i in range(GRP):
                        nc.tensor.matmul(
                            o_t,
                            lhsT=v_sb[:, GRP * g + i, :],
                            rhs=pt[:, STR * i : STR * (i + 1)],
                            start=(k == 0 and i == 0),
                            stop=(k == ngrp - 1 and i == GRP - 1),
                        )
                    # 4 row-sum matmuls packed into the 4 column groups of the
                    # PE array (concurrent on separate XBUSes)
                    for j in range(GRP):
                        nc.tensor.matmul(
                            l_t[32 * j : 32 * j + 1, :],
                            lhsT=pmk_sb[:, GRP * g + j : GRP * g + j + 1],
                            rhs=pt[:, STR * j : STR * (j + 1)],
                            start=(k == 0 and j == 0),
                            stop=(k == ngrp - 1 and j == GRP - 1),
                            tile_position=(0, 32 * j),
                            skip_group_check=True,
                        )
                while qi < len(qwork):
                    qwork[qi]()
                    qi += 1
                # evacuate unnormalized O^T and the 4 packed row-sum rows
                o_sb = osbp.tile([128, STR], F32, tag="osb", name=f"osb{c}")
                nc.vector.tensor_copy(o_sb, o_t)
                nc.gpsimd.dma_start(ot_e[c], o_sb)
                l_big = lbp.tile([128, STR], F32, tag="lbig", name=f"lbig{c}")
                nc.vector.tensor_copy(l_big, l_t)
                nc.gpsimd.dma_start(lb_e[c], l_big[0:128:32, :])

    nc.compile()
    return nc


def _tri_masks():
    """T1[k,q] = k<=q, T2[k,q] = k+128<=q for the diagonal supertile."""
    k = np.arange(128)[:, None]
    q = np.arange(STR)[None, :]
    t1 = (k <= q).astype(np.float32)
    t2 = (k + 128 <= q).astype(np.float32)
    return t1, t2


def shard_inputs(x, padding_mask, Wq, bq, Wk, bk, Wv, bv):
    """Build per-core in_maps plus the info needed to gather on the host."""
    x = np.asarray(x, np.float32)
    pm = np.asarray(padding_mask, np.float32)
    w_tiles = {}
    for name, W in (("wq", Wq), ("wk", Wk), ("wv", Wv)):
        w_tiles[name] = np.ascontiguousarray(
            np.asarray(W, np.float32).reshape(8, 128, DK).transpose(1, 0, 2)
        ).astype(BF)
    bq_t = np.ascontiguousarray(np.asarray(bq, np.float32).reshape(128, 1))
    bv_f = np.asarray(bv, np.float32)
    t1, t2 = _tri_masks()
    ones = np.ones((128, STR), np.float32)
    zeros = np.zeros((128, STR), np.float32)
    tri_g = {
        0: np.concatenate([ones, ones, t1, t2], axis=1).astype(BF),
        1: np.concatenate([zeros, zeros, t1, t2], axis=1).astype(BF),
    }
    in_maps = []
    row_maps = []
    base = np.arange(S).reshape(8, 2, STR)
    for cc in range(NCORE):
        b, g = cc % 4, cc // 4
        perm = (base[:, ::-1, :] if g == 1 else base).reshape(-1)
        xp = x[b][perm]                       # [S, D] permuted rows
        # [128, mc, pos] -> [128, chunk, mc, 512] (chunk-contiguous per partition)
        xt3 = xp.T.reshape(8, 128, S).transpose(1, 0, 2)
        xt = np.ascontiguousarray(
            xt3.reshape(128, 8, 8, 512).swapaxes(1, 2)
        ).astype(BF)
        qrows = perm.reshape(8, 2, STR)[:, 1, :]   # own rows per slot [8, 256]
        alive = pm[b][qrows].astype(np.float32)    # [8, 256]
        pmk = pm[b][perm].reshape(NKB, 128).T      # [128, 32]
        in_maps.append({
            "xt": xt,
            **w_tiles,
            "bq": bq_t,
            "pmk": np.ascontiguousarray(pmk).astype(BF),
            "tri": np.ascontiguousarray(tri_g[g]),
        })
        row_maps.append((b, qrows, alive, bv_f))
    return in_maps, row_maps


def gather_outputs(results, row_maps):
    full = np.zeros((B, S, DK), np.float32)
    for cc in range(NCORE):
        b, qrows, alive, bv_f = row_maps[cc]
        ot = np.asarray(results[cc]["ot"], np.float32)   # [8, 128, 256] O^T
        lb = np.asarray(results[cc]["lb"], np.float32)   # [8, 4, 256]
        l = lb.sum(axis=1)                               # [8, 256]
        denom = l + (alive <= 0.0)                       # dead rows: avoid /0
        o = ot.transpose(0, 2, 1) / denom[..., None] + bv_f
        o *= alive[..., None]
        full[b, qrows] = o
    return full


_NC_CACHE = None


def _get_graph():
    global _NC_CACHE
    if _NC_CACHE is None:
        _NC_CACHE = build_graph()
    return _NC_CACHE


def kernel(x, padding_mask, Wq, bq, Wk, bk, Wv, bv):
    nc = _get_graph()
    in_maps, row_maps = shard_inputs(x, padding_mask, Wq, bq, Wk, bk, Wv, bv)
    res = run_bass_kernel_spmd(nc, in_maps, core_ids=list(range(NCORE)))
    return gather_outputs(res.results, row_maps)
